# revision 42
# baseline (speedup 1.0000x reference)
"""Trainium2 Bass kernel for nn_GammaCapsGraph (capsule routing over gram matrix).

Math (per batch, X = x[b] of shape (D=128, N=1024)):
  G = X^T X (symmetric gram), u_norm = sqrt(diag G), u_hat_norm = ||G row||
  U = alpha * G rowwise, alpha = min(u_hat_norm, u_norm)/u_hat_norm
  3 routing iterations where c is a per-row scalar, so all row reductions
  collapse onto precomputed row stats:
    q[n] = min(u_hat_norm,u_norm)^2, rr[n] = alpha*bias_n*rowsum(G),
    bb[n] = N*bias_n^2        (bias verified row-constant on host)
    sq = c^2 q + 2c rr + bb;  f = sqrt(sq)/(1+sq)
    d^2 = f*(f*sq - 2(cq+rr)) + q;  d_o = global mean(d) -> t -> c' = softmax(t d)
  Output v = (f*c*alpha) * G + (f*bias_n) -- gram computed in bf16 on PE,
  fused scale+add evicts PSUM -> fp16 SBUF, DMA'd to HBM (host upcasts).

Row stats in O(N*D^2) on PE (bf16): Y = X X^T via host-pretransposed xT;
zrs_c = x_c^T [Y | svec] gives zT rows (ssq via per-chunk mult+reduce on
DVE) and rowsum(G). diag/svec/xT/bf16-cast are host-side input prep.

Act engine notes: table choice is static per function (first table set
containing it), so Sqrt ops are batched and small copies live on DVE to
minimize Sqrt<->Exp table reloads. A dummy AllReduce at t=0 pre-pays
collective rendezvous; each real AllReduce is split into start (DMA +
collective) and finish (tiny scalar chain) so PE transposes + early gram
tiles fill the wait window.

Sharding: batch 32 -> 8 cores x 4. Only cross-core data: scalar sum(d) after
iterations 0 and 1 -> two tiny AllReduces.
"""
import os

import ml_dtypes
import numpy as np

import concourse.bass as bass
import concourse.bacc as bacc
import concourse.tile as tile
import concourse.mybir as mybir
from concourse.bass_utils import run_bass_kernel_spmd

N_CORES = 8
B_LOC = 4
D = 128
N = 1024
NCH = 8  # column chunks of 128
P_P = 0.9
NUM_SECONDARY = 1024
EPS = 1e-12
T_NUM = float(np.log(P_P * (NUM_SECONDARY - 1)) - np.log(1.0 - P_P))

F = mybir.dt.float32
BF = mybir.dt.bfloat16
F16 = mybir.dt.float16
AF = mybir.ActivationFunctionType
OP = mybir.AluOpType
AX = mybir.AxisListType

LAST_EXEC_NS = None
_NC_CACHE = None


def _build():
    sim_mode = os.environ.get("KERNEL_SIM_MODE") == "1"
    mean_div = 4096.0 if sim_mode else 32768.0  # sim runs 1 core / 4 batches
    nc = bacc.Bacc("TRN2", target_bir_lowering=False, debug=False,
                   enable_asserts=False,
                   num_devices=1 if sim_mode else N_CORES)
    xbs = nc.dram_tensor("xb", (B_LOC, D, N), BF, kind="ExternalInput").ap()
    xTs = nc.dram_tensor("xT", (B_LOC, D, N), BF, kind="ExternalInput").ap()
    sv_in = nc.dram_tensor("sv", (D, B_LOC), BF, kind="ExternalInput").ap()
    diag_in = nc.dram_tensor("diag", (D, 32), F, kind="ExternalInput").ap()
    iden = nc.dram_tensor("iden", (D, D), F, kind="ExternalInput").ap()
    m8_in = nc.dram_tensor("m8", (32, 4), F, kind="ExternalInput").ap()
    m8t_in = nc.dram_tensor("m8t", (4, 32), F, kind="ExternalInput").ap()
    b32_in = nc.dram_tensor("b32", (32, D), F, kind="ExternalInput").ap()
    bb32_in = nc.dram_tensor("bb32", (32, D), F, kind="ExternalInput").ap()
    bcol_in = nc.dram_tensor("bcol", (D, 32), F, kind="ExternalInput").ap()
    bbcol_in = nc.dram_tensor("bbcol", (D, 32), F, kind="ExternalInput").ap()
    vout = nc.dram_tensor("v", (B_LOC, N, N), F16, kind="ExternalOutput").ap()

    with tile.TileContext(nc) as tc:
        with (
            tc.tile_pool(name="const", bufs=1) as cpool,
            tc.tile_pool(name="persist", bufs=1) as pp,
            tc.tile_pool(name="scr", bufs=2) as scr,
            tc.tile_pool(name="row", bufs=2) as row,
            tc.tile_pool(name="vst", bufs=10) as vst,
            tc.tile_pool(name="psb", bufs=3, space="PSUM") as psb,
            tc.tile_pool(name="pss", bufs=2, space="PSUM") as pss,
            tc.tile_pool(name="dram", bufs=1, space="DRAM") as dram,
        ):
            _cnt = [0]

            def _nm(tag):
                _cnt[0] += 1
                return f"{tag}_{_cnt[0]}"

            # ---- persistent tiles ----
            xb_t = [pp.tile([D, N], BF, tag=f"xb{b}", name=f"xb{b}")
                    for b in range(B_LOC)]
            xT_t = [pp.tile([D, N], BF, tag=f"xT{b}", name=f"xT{b}")
                    for b in range(B_LOC)]
            diag_pack = pp.tile([D, 32], F)   # col 8b+c: diag G, n=128c+p
            ssq_pack = pp.tile([D, 32], F)    # ||G row||^2
            rsum_pack = pp.tile([D, 32], F)   # rowsum(G)
            q_pack = pp.tile([D, 32], F)
            rr_pack = pp.tile([D, 32], F)
            alpha_pack = pp.tile([D, 32], F)
            d0_pack = pp.tile([D, 32], F)

            # dummy collective first: staging DMA leads the sync ring so
            # the rendezvous warmup fires at t~1us, hidden under phase 1
            wstg = cpool.tile([1, 8], F)
            nc.vector.memset(wstg[:], 0.0)
            wain = dram.tile([1, 8], F, tag="warmin", name="dr_wain")
            waout = dram.tile([1, 8], F, tag="warmout", addr_space="Shared",
                              name="dr_waout")
            nc.sync.dma_start(wain[:], wstg[:])
            if sim_mode:
                nc.sync.dma_start(waout[:], wain[:])
            else:
                nc.gpsimd.collective_compute(
                    "AllReduce", OP.add,
                    replica_groups=[list(range(N_CORES))],
                    ins=[wain.opt()], outs=[waout.opt()],
                )

            # input DMAs (phase-1 critical), constants behind them
            for b in range(B_LOC):
                nc.sync.dma_start(xb_t[b][:], xbs[b])
                nc.scalar.dma_start(xT_t[b][:], xTs[b])

            # warm the default (exp-family) act table at t=0
            warm = cpool.tile([1, 1], F)
            nc.vector.memset(warm[:], 1.0)
            warm_o = cpool.tile([1, 1], F)
            nc.scalar.activation(warm_o[:], warm[:], AF.Square)

            sv4 = cpool.tile([D, B_LOC], BF)
            nc.scalar.dma_start(sv4[:], sv_in[:])
            ident = cpool.tile([D, D], F)
            nc.scalar.dma_start(ident[:], iden[:])
            ones128 = cpool.tile([D, 1], F)
            nc.vector.memset(ones128[:], 1.0)
            ones1x32 = cpool.tile([1, 32], F)
            nc.vector.memset(ones1x32[:], 1.0)
            ones32 = cpool.tile([32, 1], F)
            nc.vector.memset(ones32[:], 1.0)
            m8 = cpool.tile([32, 4], F)       # m8[p,b] = 1 if p//8==b
            nc.scalar.dma_start(m8[:], m8_in[:])
            m8t = cpool.tile([4, 32], F)      # transpose of m8
            nc.scalar.dma_start(m8t[:], m8t_in[:])
            b32 = cpool.tile([32, D], F)      # row-layout bias
            nc.scalar.dma_start(b32[:], b32_in[:])
            bb32 = cpool.tile([32, D], F)     # N * bias^2, row layout
            nc.scalar.dma_start(bb32[:], bb32_in[:])
            bcol = cpool.tile([D, 32], F)     # bias col-layout, x4 batches
            nc.scalar.dma_start(bcol[:], bcol_in[:])
            bbcol = cpool.tile([D, 32], F)    # N * bias^2 col-layout
            nc.scalar.dma_start(bbcol[:], bbcol_in[:])
            nc.scalar.dma_start(diag_pack[:], diag_in[:])

            # ================= phase 1: stats per batch =================
            for b in range(B_LOC):
                xb = xb_t[b]
                xT = xT_t[b]
                # Y = X X^T, accumulate 8 chunks (bf16 PE)
                yps = pss.tile([D, D], F, tag="small", name=_nm("ps_yps"))
                for c in range(NCH):
                    sl = slice(128 * c, 128 * (c + 1))
                    nc.tensor.matmul(yps[:], xT[:, sl], xT[:, sl],
                                     start=(c == 0), stop=(c == NCH - 1))
                # Yb = [Y | svec] bf16 for the zT matmuls
                yb = scr.tile([D, D + 1], BF, tag="yb")
                nc.scalar.copy(yb[:, 0:D], yps[:])
                nc.vector.tensor_copy(yb[:, D:D + 1], sv4[:, b:b + 1])
                # zrs_c = x_c^T [Y | svec]: cols 0..127 = (X^T Y) chunk rows,
                # col 128 = rowsum(G). Grouped 3 chunks per PSUM tile
                # (136-float stride keeps matmul dsts 32B-aligned) so the
                # mult/reduce/copy extraction runs once per group.
                xz = scr.tile([D, 3, D], BF, tag="xz")
                for g, (c0, cnt) in enumerate(((0, 3), (3, 3), (6, 2))):
                    zg = pss.tile([D, 3, 136], F, tag="small", name=_nm("ps_zg"))
                    for j in range(cnt):
                        c = c0 + j
                        nc.tensor.matmul(zg[:, j, 0:D + 1],
                                         xb[:, 128 * c:128 * (c + 1)], yb[:],
                                         start=True, stop=True)
                    k = 8 * b + c0
                    nc.vector.tensor_tensor(
                        xz[:, 0:cnt, :],
                        xT[:, 128 * c0:128 * (c0 + cnt)].rearrange(
                            "p (c j) -> p c j", j=D),
                        zg[:, 0:cnt, 0:D], op=OP.mult)
                    nc.vector.reduce_sum(ssq_pack[:, k:k + cnt],
                                         xz[:, 0:cnt, :], axis=AX.X)
                    nc.vector.tensor_copy(rsum_pack[:, k:k + cnt],
                                          zg[:, 0:cnt, D:D + 1])

            # ===== gram early: all 32 tiles, evicted UNSCALED to fp16 SBUF.
            # Everything until the second AllReduce resolves is pinned by the
            # collective engine's fixed init (~55us from warmup trigger), so
            # gram work is free if it hides there; only the final
            # scale+DMA pass remains on the critical tail.
            gsb = [pp.tile([D, N], F16, tag=f"g{b}_{ch}", name=f"g{b}_{ch}")
                   for b in range(B_LOC) for ch in range(NCH)]
            for b in range(B_LOC):
                for ch in range(NCH):
                    gps = psb.tile([D, N], F, tag="big")
                    lhs = xb_t[b][:, 128 * ch:128 * (ch + 1)]
                    nc.tensor.matmul(gps[:, 0:512], lhs, xb_t[b][:, 0:512],
                                     start=True, stop=True)
                    nc.tensor.matmul(gps[:, 512:1024], lhs,
                                     xb_t[b][:, 512:1024],
                                     start=True, stop=True)
                    dst = gsb[b * NCH + ch]
                    if ch % 2 == 0:
                        nc.scalar.copy(dst[:], gps[:])
                    else:
                        nc.vector.tensor_copy(dst[:], gps[:])

            # ===== derived stats + iteration 0, batched (128,32) =====
            # (all Sqrt ops adjacent on Act; everything else on DVE)
            un32 = scr.tile([D, 32], F, tag="un32")
            nc.scalar.activation(un32[:], diag_pack[:], AF.Sqrt)
            uh32 = scr.tile([D, 32], F, tag="uh32")
            nc.scalar.activation(uh32[:], ssq_pack[:], AF.Sqrt)
            nn32 = scr.tile([D, 32], F, tag="nn32")
            nc.vector.tensor_tensor(nn32[:], uh32[:], un32[:], op=OP.min)
            nc.vector.tensor_tensor(q_pack[:], nn32[:], nn32[:], op=OP.mult)
            ivh = scr.tile([D, 32], F, tag="ivh")
            nc.vector.reciprocal(ivh[:], uh32[:])
            nc.vector.tensor_tensor(alpha_pack[:], nn32[:], ivh[:], op=OP.mult)
            t1c = scr.tile([D, 32], F, tag="t1c")
            nc.vector.tensor_tensor(t1c[:], alpha_pack[:], rsum_pack[:], op=OP.mult)
            nc.vector.tensor_tensor(rr_pack[:], t1c[:], bcol[:], op=OP.mult)
            # --- iteration 0 (c = 1/N) in column layout ---
            c0 = 1.0 / N
            sqc = scr.tile([D, 32], F, tag="sqc")
            nc.vector.scalar_tensor_tensor(sqc[:], q_pack[:], c0 * c0, bbcol[:],
                                           op0=OP.mult, op1=OP.add)
            nc.vector.scalar_tensor_tensor(sqc[:], rr_pack[:], 2.0 * c0, sqc[:],
                                           op0=OP.mult, op1=OP.add)
            # m = c0*q + rr
            mc = scr.tile([D, 32], F, tag="mc")
            nc.vector.scalar_tensor_tensor(mc[:], q_pack[:], c0, rr_pack[:],
                                           op0=OP.mult, op1=OP.add)
            sqsc = scr.tile([D, 32], F, tag="sqsc")
            nc.scalar.activation(sqsc[:], sqc[:], AF.Sqrt)
            denc = scr.tile([D, 32], F, tag="denc")
            nc.vector.tensor_scalar_add(denc[:], sqc[:], 1.0)
            invc = scr.tile([D, 32], F, tag="invc")
            nc.vector.reciprocal(invc[:], denc[:])
            fcl = scr.tile([D, 32], F, tag="fcl")
            nc.vector.tensor_tensor(fcl[:], sqsc[:], invc[:], op=OP.mult)
            # d^2 = f*(f*sq - 2m) + q
            d2c = scr.tile([D, 32], F, tag="d2c")
            nc.vector.tensor_tensor(d2c[:], fcl[:], sqc[:], op=OP.mult)
            nc.vector.scalar_tensor_tensor(d2c[:], mc[:], -2.0, d2c[:],
                                           op0=OP.mult, op1=OP.add)
            nc.vector.tensor_tensor(d2c[:], d2c[:], fcl[:], op=OP.mult)
            nc.vector.tensor_tensor(d2c[:], d2c[:], q_pack[:], op=OP.add)
            d0p1 = row.tile([D, 1], F, tag="d0p1", name="d0p1")
            nc.scalar.activation(d0_pack[:], d2c[:], AF.Sqrt, accum_out=d0p1[:])

            # ================= routing helpers =================
            def row_t(tag):
                return row.tile([32, D], F, tag=tag, name=_nm(tag))

            def ar_start(tot, idx):
                """stage tot (1,1) -> DRAM -> AllReduce -> DMA result back"""
                stg = row.tile([1, 8], F, tag="arstg", name=_nm("arstg"))
                nc.vector.memset(stg[:], 0.0)
                nc.vector.tensor_copy(stg[0:1, 0:1], tot[:])
                ain = dram.tile([1, 8], F, tag=f"arin{idx}", name=_nm("dr_ain"))
                aout = dram.tile([1, 8], F, tag=f"arout{idx}",
                                 addr_space="Shared", name=_nm("dr_aout"))
                nc.sync.dma_start(ain[:], stg[:])
                if sim_mode:
                    nc.sync.dma_start(aout[:], ain[:])
                else:
                    nc.gpsimd.collective_compute(
                        "AllReduce", OP.add,
                        replica_groups=[list(range(N_CORES))],
                        ins=[ain.opt()], outs=[aout.opt()],
                    )
                gsum = row.tile([1, 1], F, tag="gsum", name=_nm("gsum"))
                nc.sync.dma_start(gsum[:], aout[0:1, 0:1])
                return gsum

            def ar_finish(gsum):
                """gsum -> t scalar -> (32,1) broadcast"""
                dent = row.tile([1, 1], F, tag="dent", name=_nm("dent"))
                nc.vector.tensor_scalar(dent[:], gsum[:], -0.5 / mean_div, EPS,
                                        op0=OP.mult, op1=OP.add)
                it = row.tile([1, 1], F, tag="it", name=_nm("it"))
                nc.vector.reciprocal(it[:], dent[:])
                tv = row.tile([1, 1], F, tag="tv", name=_nm("tv"))
                nc.vector.tensor_scalar_mul(tv[:], it[:], T_NUM)
                tb_ps = pss.tile([32, 1], F, tag="small", name=_nm("ps_tb"))
                nc.tensor.matmul(tb_ps[:], ones1x32[:], tv[:], start=True, stop=True)
                tb = row.tile([32, 1], F, tag="tb", name=_nm("tb"))
                nc.vector.tensor_copy(tb[:], tb_ps[:])
                return tb

            def softmax_c(d, tb):
                e = row_t("e")
                part = row.tile([32, 1], F, tag="epart", name=_nm("epart"))
                nc.scalar.activation(e[:], d[:], AF.Exp, scale=tb[:],
                                     accum_out=part[:])
                p4 = pss.tile([4, 1], F, tag="small", name=_nm("ps_p4"))
                nc.tensor.matmul(p4[:], m8[:], part[:], start=True, stop=True)
                s4 = row.tile([4, 1], F, tag="s4", name=_nm("s4"))
                nc.vector.tensor_copy(s4[:], p4[:])
                i4 = row.tile([4, 1], F, tag="i4", name=_nm("i4"))
                nc.vector.reciprocal(i4[:], s4[:])
                p32 = pss.tile([32, 1], F, tag="small", name=_nm("ps_p32"))
                nc.tensor.matmul(p32[:], m8t[:], i4[:], start=True, stop=True)
                inv32 = row.tile([32, 1], F, tag="inv32", name=_nm("inv32"))
                nc.vector.tensor_copy(inv32[:], p32[:])
                c = row_t("c")
                nc.vector.tensor_scalar(c[:], e[:], inv32[:], None, op0=OP.mult)
                return c

            def iter_sq(c):
                """sq = c(cq + 2rr) + bb; also m = cq + rr for d^2"""
                t1r = row_t("t1r")
                nc.vector.tensor_tensor(t1r[:], c[:], q_stack[:], op=OP.mult)
                m = row_t("m")
                nc.vector.tensor_tensor(m[:], t1r[:], rr_stack[:], op=OP.add)
                nc.vector.scalar_tensor_tensor(t1r[:], rr_stack[:], 2.0, t1r[:],
                                               op0=OP.mult, op1=OP.add)
                sq = row_t("sq")
                nc.vector.tensor_tensor(sq[:], c[:], t1r[:], op=OP.mult)
                nc.vector.tensor_tensor(sq[:], sq[:], bb32[:], op=OP.add)
                return sq, m

            def compute_f(sq):
                sqs = row_t("sqs")
                nc.scalar.activation(sqs[:], sq[:], AF.Sqrt)
                den = row_t("den")
                nc.vector.tensor_scalar_add(den[:], sq[:], 1.0)
                inv = row_t("invd")
                nc.vector.reciprocal(inv[:], den[:])
                f = row_t("f")
                nc.vector.tensor_tensor(f[:], sqs[:], inv[:], op=OP.mult)
                return f

            def compute_d_sum(f, m, sq):
                """d = sqrt(f(f sq - 2m) + q), plus fused row-sum of d"""
                d2 = row_t("d2")
                nc.vector.tensor_tensor(d2[:], f[:], sq[:], op=OP.mult)
                nc.vector.scalar_tensor_tensor(d2[:], m[:], -2.0, d2[:],
                                               op0=OP.mult, op1=OP.add)
                nc.vector.tensor_tensor(d2[:], d2[:], f[:], op=OP.mult)
                nc.vector.tensor_tensor(d2[:], d2[:], q_stack[:], op=OP.add)
                d = row_t("d")
                part = row.tile([32, 1], F, tag="dpart", name=_nm("dpart"))
                nc.scalar.activation(d[:], d2[:], AF.Sqrt, accum_out=part[:])
                tot = pss.tile([1, 1], F, tag="small", name=_nm("ps_tot"))
                nc.tensor.matmul(tot[:], part[:], ones32[:], start=True, stop=True)
                return d, tot

            # iteration 0: AllReduce the global d sum; fill the wait window
            # with layout transposes and the first gram tiles
            d0tot = pss.tile([1, 1], F, tag="small", name="ps_d0tot")
            nc.tensor.matmul(d0tot[:], d0p1[:], ones128[:], start=True, stop=True)
            gsum0 = ar_start(d0tot, 1)

            qs_ps = pss.tile([32, D], F, tag="small", name=_nm("ps_qs"))
            nc.tensor.transpose(qs_ps[:], q_pack[:], ident[:])
            q_stack = pp.tile([32, D], F)
            nc.vector.tensor_copy(q_stack[:], qs_ps[:])
            rs_ps = pss.tile([32, D], F, tag="small", name=_nm("ps_rs"))
            nc.tensor.transpose(rs_ps[:], rr_pack[:], ident[:])
            rr_stack = pp.tile([32, D], F)
            nc.vector.tensor_copy(rr_stack[:], rs_ps[:])
            d0r_ps = pss.tile([32, D], F, tag="small", name=_nm("ps_d0r"))
            nc.tensor.transpose(d0r_ps[:], d0_pack[:], ident[:])
            d = row_t("d0row")
            nc.vector.tensor_copy(d[:], d0r_ps[:])


            # ================= routing iterations =================
            tb = ar_finish(gsum0)
            # iteration 1
            c = softmax_c(d, tb)
            sq, m = iter_sq(c)
            f = compute_f(sq)
            d, tot = compute_d_sum(f, m, sq)
            gsum1 = ar_start(tot, 2)
            nc.scalar.activation(warm_o[:], warm[:], AF.Exp)  # prefetch table
            tb = ar_finish(gsum1)
            # iteration 2 (final): only need c, f
            c = softmax_c(d, tb)
            sq, _m = iter_sq(c)
            f = compute_f(sq)
            fc = row_t("fc")
            nc.vector.tensor_tensor(fc[:], f[:], c[:], op=OP.mult)

            # row -> column layout: (32,128) -> (128,32)
            fcT_ps = pss.tile([D, 32], F, tag="small", name=_nm("ps_fcT"))
            nc.tensor.transpose(fcT_ps[:], fc[:], ident[0:32, 0:32])
            fcT = pp.tile([D, 32], F)
            nc.vector.tensor_copy(fcT[:], fcT_ps[:])
            fT_ps = pss.tile([D, 32], F, tag="small", name=_nm("ps_fT"))
            nc.tensor.transpose(fT_ps[:], f[:], ident[0:32, 0:32])
            fT = pp.tile([D, 32], F)
            nc.vector.tensor_copy(fT[:], fT_ps[:])

            # evict coefficients for all batches at once
            acol = pp.tile([D, 32], F)
            nc.vector.tensor_tensor(acol[:], fcT[:], alpha_pack[:], op=OP.mult)
            ccol = pp.tile([D, 32], F)
            nc.vector.tensor_tensor(ccol[:], fT[:], bcol[:], op=OP.mult)

            # ===== phase 3: v = A*G + C from fp16 SBUF gram, stream out =====
            vt_cur = [None]
            for b in range(B_LOC):
                for ch in range(NCH):
                    g, h = ch // 2, ch % 2
                    if h == 0:
                        vt_cur[0] = vst.tile([D, 2 * N], F16, tag="vt",
                                             name=_nm("vt"))
                    vt = vt_cur[0]
                    dst = vt[:, N * h:N * (h + 1)]
                    k = 8 * b + ch
                    src = gsb[b * NCH + ch]
                    # DVE runs fp16 tiles at 2x; give it 3x the tiles of Act
                    if ch in (2, 5, 7) and b % 2 == 0 or ch in (2, 6) and b % 2 == 1:
                        nc.scalar.activation(dst, src[:], AF.Identity,
                                             bias=ccol[:, k:k + 1],
                                             scale=acol[:, k:k + 1])
                    else:
                        nc.vector.tensor_scalar(dst, src[:], acol[:, k:k + 1],
                                                ccol[:, k:k + 1],
                                                op0=OP.mult, op1=OP.add)
                    if h == 1:
                        ring = (nc.sync, nc.gpsimd, nc.scalar)[(4 * b + g) % 3]
                        dst_ap = vout[b, 256 * g:256 * (g + 1), :].rearrange(
                            "(s p) n -> p s n", p=D)
                        src_ap = vt[:].rearrange("p (s n) -> p s n", n=N)
                        ring.dma_start(dst_ap, src_ap)

    nc.compile()
    return nc


def _get_nc():
    global _NC_CACHE
    if _NC_CACHE is None:
        _NC_CACHE = _build()
    return _NC_CACHE


def _reference_numpy(x, bias):
    """General fallback (non-row-constant bias): straight numpy port."""
    x = x.astype(np.float32)
    bias = bias.astype(np.float32)
    u_norm = np.linalg.norm(x, axis=1)[..., None]
    u_hat = np.einsum('bdn,bdm->bnm', x, x)
    u_hat_norm = np.linalg.norm(u_hat, axis=-1, keepdims=True)
    new_norm = np.minimum(u_hat_norm, u_norm)
    u_hat = u_hat / u_hat_norm * new_norm
    t_num = np.float32(T_NUM)
    b_ij = np.zeros(u_hat.shape, dtype=np.float32)
    v_j = None
    for it in range(3):
        m = b_ij.max(axis=1, keepdims=True)
        e = np.exp(b_ij - m)
        c_ij = e / e.sum(axis=1, keepdims=True)
        s_j = c_ij * u_hat + bias
        sqn = np.sum(s_j * s_j, axis=-1, keepdims=True)
        v_j = sqn * s_j / ((1.0 + sqn) * np.sqrt(sqn))
        if it < 2:
            dd = np.linalg.norm(v_j - u_hat, axis=-1, keepdims=True)
            d_o = dd.mean()
            t = t_num / (0.5 * d_o - d_o + EPS)
            b_ij = t * dd
    return v_j


def kernel(x, bias):
    global LAST_EXEC_NS
    x = np.ascontiguousarray(x, dtype=np.float32)
    bias = np.ascontiguousarray(bias, dtype=np.float32)
    B = x.shape[0]
    row_const = bool((bias == bias[:, :, :1]).all())
    if not row_const or B != 32 or x.shape[1:] != (D, N):
        return _reference_numpy(x, bias)
    brow = np.ascontiguousarray(bias[0, :, 0])  # (N,)
    xb16 = x.astype(ml_dtypes.bfloat16)
    # xT[b, p, 128c+j] = x[b, j, 128c+p]  (chunkwise transpose)
    xT16 = np.ascontiguousarray(
        x.reshape(B, D, NCH, D).transpose(0, 3, 2, 1)
    ).reshape(B, D, N).astype(ml_dtypes.bfloat16)
    sv = x.sum(axis=2).astype(ml_dtypes.bfloat16)  # (B, D) row sums
    # diag[b, p, c] col-layout: sum_d x[b,d,128c+p]^2 -> (B, 128, 8)
    dsq = (x * x).sum(axis=1).reshape(B, NCH, D).transpose(0, 2, 1)
    iden = np.eye(D, dtype=np.float32)
    m8 = np.zeros((32, 4), dtype=np.float32)
    m8t = np.zeros((4, 32), dtype=np.float32)
    for b in range(4):
        m8[8 * b:8 * b + 8, b] = 1.0
        m8t[b, 8 * b:8 * b + 8] = 1.0
    b32 = np.ascontiguousarray(np.tile(brow.reshape(8, 128), (4, 1)))
    bb32 = np.ascontiguousarray(np.float32(N) * b32 * b32)
    bcol = np.ascontiguousarray(np.tile(brow.reshape(8, 128).T, (1, 4)))
    bbcol = np.ascontiguousarray(np.float32(N) * bcol * bcol)
    nc = _get_nc()
    in_maps = [
        {"xb": np.ascontiguousarray(xb16[4 * c:4 * c + 4]),
         "xT": np.ascontiguousarray(xT16[4 * c:4 * c + 4]),
         "sv": np.ascontiguousarray(sv[4 * c:4 * c + 4].T),
         "diag": np.ascontiguousarray(
             dsq[4 * c:4 * c + 4].transpose(1, 0, 2).reshape(D, 32)),
         "iden": iden, "m8": m8, "m8t": m8t, "b32": b32, "bb32": bb32,
         "bcol": bcol, "bbcol": bbcol}
        for c in range(N_CORES)
    ]
    res = run_bass_kernel_spmd(nc, in_maps, core_ids=list(range(N_CORES)))
    LAST_EXEC_NS = res.exec_time_ns
    return np.concatenate(
        [res.results[c]["v"].astype(np.float32) for c in range(N_CORES)], axis=0)


# revision 43
# speedup vs baseline: 1.0513x; 1.0513x over previous
"""Trainium2 Bass kernel for nn_GammaCapsGraph (capsule routing over gram matrix).

Math (per batch, X = x[b] of shape (D=128, N=1024)):
  G = X^T X (symmetric gram), u_norm = sqrt(diag G), u_hat_norm = ||G row||
  U = alpha * G rowwise, alpha = min(u_hat_norm, u_norm)/u_hat_norm
  3 routing iterations where c is a per-row scalar, so all row reductions
  collapse onto precomputed row stats:
    q[n] = min(u_hat_norm,u_norm)^2, rr[n] = alpha*bias_n*rowsum(G),
    bb[n] = N*bias_n^2        (bias verified row-constant on host)
    sq = c^2 q + 2c rr + bb;  f = sqrt(sq)/(1+sq)
    d^2 = f*(f*sq - 2(cq+rr)) + q;  d_o = global mean(d) -> t -> c' = softmax(t d)
  Output v = (f*c*alpha) * G + (f*bias_n) -- gram computed in bf16 on PE,
  fused scale+add evicts PSUM -> fp16 SBUF, DMA'd to HBM (host upcasts).

Row stats in O(N*D^2) on PE (bf16): Y = X X^T via host-pretransposed xT;
zrs_c = x_c^T [Y | svec] gives zT rows (ssq via per-chunk mult+reduce on
DVE) and rowsum(G). diag/svec/xT/bf16-cast are host-side input prep.

Act engine notes: table choice is static per function (first table set
containing it), so Sqrt ops are batched and small copies live on DVE to
minimize Sqrt<->Exp table reloads. A dummy AllReduce at t=0 pre-pays
collective rendezvous; each real AllReduce is split into start (DMA +
collective) and finish (tiny scalar chain) so PE transposes + early gram
tiles fill the wait window.

Sharding: batch 32 -> 8 cores x 4. Only cross-core data: scalar sum(d) after
iterations 0 and 1 -> two tiny AllReduces.
"""
import os

import ml_dtypes
import numpy as np

import concourse.bass as bass
import concourse.bacc as bacc
import concourse.tile as tile
import concourse.mybir as mybir
from concourse.bass_utils import run_bass_kernel_spmd

N_CORES = 8
B_LOC = 4
D = 128
N = 1024
NCH = 8  # column chunks of 128
P_P = 0.9
NUM_SECONDARY = 1024
EPS = 1e-12
T_NUM = float(np.log(P_P * (NUM_SECONDARY - 1)) - np.log(1.0 - P_P))

F = mybir.dt.float32
BF = mybir.dt.bfloat16
F16 = mybir.dt.float16
AF = mybir.ActivationFunctionType
OP = mybir.AluOpType
AX = mybir.AxisListType

LAST_EXEC_NS = None
_NC_CACHE = None


def _build():
    sim_mode = os.environ.get("KERNEL_SIM_MODE") == "1"
    mean_div = 4096.0 if sim_mode else 32768.0  # sim runs 1 core / 4 batches
    nc = bacc.Bacc("TRN2", target_bir_lowering=False, debug=False,
                   enable_asserts=False,
                   num_devices=1 if sim_mode else N_CORES)
    xbs = nc.dram_tensor("xb", (B_LOC, D, N), BF, kind="ExternalInput").ap()
    xTs = nc.dram_tensor("xT", (B_LOC, D, N), BF, kind="ExternalInput").ap()
    sv_in = nc.dram_tensor("sv", (D, B_LOC), BF, kind="ExternalInput").ap()
    diag_in = nc.dram_tensor("diag", (D, 32), F, kind="ExternalInput").ap()
    iden = nc.dram_tensor("iden", (D, D), F, kind="ExternalInput").ap()
    m8_in = nc.dram_tensor("m8", (32, 4), F, kind="ExternalInput").ap()
    m8t_in = nc.dram_tensor("m8t", (4, 32), F, kind="ExternalInput").ap()
    b32_in = nc.dram_tensor("b32", (32, D), F, kind="ExternalInput").ap()
    bb32_in = nc.dram_tensor("bb32", (32, D), F, kind="ExternalInput").ap()
    bcol_in = nc.dram_tensor("bcol", (D, 32), F, kind="ExternalInput").ap()
    bbcol_in = nc.dram_tensor("bbcol", (D, 32), F, kind="ExternalInput").ap()
    vout = nc.dram_tensor("v", (B_LOC, N, N), F16, kind="ExternalOutput").ap()

    with tile.TileContext(nc) as tc:
        with (
            tc.tile_pool(name="const", bufs=1) as cpool,
            tc.tile_pool(name="persist", bufs=1) as pp,
            tc.tile_pool(name="scr", bufs=2) as scr,
            tc.tile_pool(name="row", bufs=2) as row,
            tc.tile_pool(name="vst", bufs=6) as vst,
            tc.tile_pool(name="psb", bufs=3, space="PSUM") as psb,
            tc.tile_pool(name="pss", bufs=2, space="PSUM") as pss,
            tc.tile_pool(name="dram", bufs=1, space="DRAM") as dram,
        ):
            _cnt = [0]

            def _nm(tag):
                _cnt[0] += 1
                return f"{tag}_{_cnt[0]}"

            # ---- persistent tiles ----
            xb_t = [pp.tile([D, N], BF, tag=f"xb{b}", name=f"xb{b}")
                    for b in range(B_LOC)]
            xT_t = [pp.tile([D, N], BF, tag=f"xT{b}", name=f"xT{b}")
                    for b in range(B_LOC)]
            diag_pack = pp.tile([D, 32], F)   # col 8b+c: diag G, n=128c+p
            ssq_pack = pp.tile([D, 32], F)    # ||G row||^2
            rsum_pack = pp.tile([D, 32], F)   # rowsum(G)
            q_pack = pp.tile([D, 32], F)
            rr_pack = pp.tile([D, 32], F)
            alpha_pack = pp.tile([D, 32], F)
            d0_pack = pp.tile([D, 32], F)

            # dummy collective first: staging DMA leads the sync ring so
            # the rendezvous warmup fires at t~1us, hidden under phase 1
            wstg = cpool.tile([1, 8], F)
            nc.vector.memset(wstg[:], 0.0)
            wain = dram.tile([1, 8], F, tag="warmin", name="dr_wain")
            waout = dram.tile([1, 8], F, tag="warmout", addr_space="Shared",
                              name="dr_waout")
            nc.sync.dma_start(wain[:], wstg[:])
            if sim_mode:
                nc.sync.dma_start(waout[:], wain[:])
            else:
                nc.gpsimd.collective_compute(
                    "AllReduce", OP.add,
                    replica_groups=[list(range(N_CORES))],
                    ins=[wain.opt()], outs=[waout.opt()],
                )

            # input DMAs (phase-1 critical), constants behind them
            for b in range(B_LOC):
                nc.sync.dma_start(xb_t[b][:], xbs[b])
                nc.scalar.dma_start(xT_t[b][:], xTs[b])

            # warm the default (exp-family) act table at t=0
            warm = cpool.tile([1, 1], F)
            nc.vector.memset(warm[:], 1.0)
            warm_o = cpool.tile([1, 1], F)
            nc.scalar.activation(warm_o[:], warm[:], AF.Square)

            sv4 = cpool.tile([D, B_LOC], BF)
            nc.scalar.dma_start(sv4[:], sv_in[:])
            ident = cpool.tile([D, D], F)
            nc.scalar.dma_start(ident[:], iden[:])
            ones128 = cpool.tile([D, 1], F)
            nc.vector.memset(ones128[:], 1.0)
            ones1x32 = cpool.tile([1, 32], F)
            nc.vector.memset(ones1x32[:], 1.0)
            ones32 = cpool.tile([32, 1], F)
            nc.vector.memset(ones32[:], 1.0)
            m8 = cpool.tile([32, 4], F)       # m8[p,b] = 1 if p//8==b
            nc.scalar.dma_start(m8[:], m8_in[:])
            m8t = cpool.tile([4, 32], F)      # transpose of m8
            nc.scalar.dma_start(m8t[:], m8t_in[:])
            b32 = cpool.tile([32, D], F)      # row-layout bias
            nc.scalar.dma_start(b32[:], b32_in[:])
            bb32 = cpool.tile([32, D], F)     # N * bias^2, row layout
            nc.scalar.dma_start(bb32[:], bb32_in[:])
            bcol = cpool.tile([D, 32], F)     # bias col-layout, x4 batches
            nc.scalar.dma_start(bcol[:], bcol_in[:])
            bbcol = cpool.tile([D, 32], F)    # N * bias^2 col-layout
            nc.scalar.dma_start(bbcol[:], bbcol_in[:])
            nc.scalar.dma_start(diag_pack[:], diag_in[:])

            # ================= phase 1: stats per batch =================
            for b in range(B_LOC):
                xb = xb_t[b]
                xT = xT_t[b]
                # Y = X X^T, accumulate 8 chunks (bf16 PE)
                yps = pss.tile([D, D], F, tag="small", name=_nm("ps_yps"))
                for c in range(NCH):
                    sl = slice(128 * c, 128 * (c + 1))
                    nc.tensor.matmul(yps[:], xT[:, sl], xT[:, sl],
                                     start=(c == 0), stop=(c == NCH - 1))
                # Yb = [Y | svec] bf16 for the zT matmuls
                yb = scr.tile([D, D + 1], BF, tag="yb")
                nc.scalar.copy(yb[:, 0:D], yps[:])
                nc.vector.tensor_copy(yb[:, D:D + 1], sv4[:, b:b + 1])
                # zrs_c = x_c^T [Y | svec]: cols 0..127 = (X^T Y) chunk rows,
                # col 128 = rowsum(G). Grouped 3 chunks per PSUM tile
                # (136-float stride keeps matmul dsts 32B-aligned) so the
                # mult/reduce/copy extraction runs once per group.
                xz = scr.tile([D, 3, D], BF, tag="xz")
                for g, (c0, cnt) in enumerate(((0, 3), (3, 3), (6, 2))):
                    zg = pss.tile([D, 3, 136], F, tag="small", name=_nm("ps_zg"))
                    for j in range(cnt):
                        c = c0 + j
                        nc.tensor.matmul(zg[:, j, 0:D + 1],
                                         xb[:, 128 * c:128 * (c + 1)], yb[:],
                                         start=True, stop=True)
                    k = 8 * b + c0
                    nc.vector.tensor_tensor(
                        xz[:, 0:cnt, :],
                        xT[:, 128 * c0:128 * (c0 + cnt)].rearrange(
                            "p (c j) -> p c j", j=D),
                        zg[:, 0:cnt, 0:D], op=OP.mult)
                    nc.vector.reduce_sum(ssq_pack[:, k:k + cnt],
                                         xz[:, 0:cnt, :], axis=AX.X)
                    nc.vector.tensor_copy(rsum_pack[:, k:k + cnt],
                                          zg[:, 0:cnt, D:D + 1])

            # ===== gram early: all 32 tiles, evicted UNSCALED to fp16 SBUF.
            # Everything until the second AllReduce resolves is pinned by the
            # collective engine's fixed init (~55us from warmup trigger), so
            # gram work is free if it hides there; only the final
            # scale+DMA pass remains on the critical tail.
            gsb = [pp.tile([D, N], F16, tag=f"g{b}_{ch}", name=f"g{b}_{ch}")
                   for b in range(B_LOC) for ch in range(NCH)]
            for b in range(B_LOC):
                for ch in range(NCH):
                    gps = psb.tile([D, N], F, tag="big")
                    lhs = xb_t[b][:, 128 * ch:128 * (ch + 1)]
                    nc.tensor.matmul(gps[:, 0:512], lhs, xb_t[b][:, 0:512],
                                     start=True, stop=True)
                    nc.tensor.matmul(gps[:, 512:1024], lhs,
                                     xb_t[b][:, 512:1024],
                                     start=True, stop=True)
                    dst = gsb[b * NCH + ch]
                    if ch % 2 == 0:
                        nc.scalar.copy(dst[:], gps[:])
                    else:
                        nc.vector.tensor_copy(dst[:], gps[:])

            # ===== derived stats + iteration 0, batched (128,32) =====
            # (all Sqrt ops adjacent on Act; everything else on DVE)
            un32 = scr.tile([D, 32], F, tag="un32")
            nc.scalar.activation(un32[:], diag_pack[:], AF.Sqrt)
            uh32 = scr.tile([D, 32], F, tag="uh32")
            nc.scalar.activation(uh32[:], ssq_pack[:], AF.Sqrt)
            nn32 = scr.tile([D, 32], F, tag="nn32")
            nc.vector.tensor_tensor(nn32[:], uh32[:], un32[:], op=OP.min)
            nc.vector.tensor_tensor(q_pack[:], nn32[:], nn32[:], op=OP.mult)
            ivh = scr.tile([D, 32], F, tag="ivh")
            nc.vector.reciprocal(ivh[:], uh32[:])
            nc.vector.tensor_tensor(alpha_pack[:], nn32[:], ivh[:], op=OP.mult)
            t1c = scr.tile([D, 32], F, tag="t1c")
            nc.vector.tensor_tensor(t1c[:], alpha_pack[:], rsum_pack[:], op=OP.mult)
            nc.vector.tensor_tensor(rr_pack[:], t1c[:], bcol[:], op=OP.mult)
            # --- iteration 0 (c = 1/N) in column layout ---
            c0 = 1.0 / N
            sqc = scr.tile([D, 32], F, tag="sqc")
            nc.vector.scalar_tensor_tensor(sqc[:], q_pack[:], c0 * c0, bbcol[:],
                                           op0=OP.mult, op1=OP.add)
            nc.vector.scalar_tensor_tensor(sqc[:], rr_pack[:], 2.0 * c0, sqc[:],
                                           op0=OP.mult, op1=OP.add)
            # m = c0*q + rr
            mc = scr.tile([D, 32], F, tag="mc")
            nc.vector.scalar_tensor_tensor(mc[:], q_pack[:], c0, rr_pack[:],
                                           op0=OP.mult, op1=OP.add)
            sqsc = scr.tile([D, 32], F, tag="sqsc")
            nc.scalar.activation(sqsc[:], sqc[:], AF.Sqrt)
            denc = scr.tile([D, 32], F, tag="denc")
            nc.vector.tensor_scalar_add(denc[:], sqc[:], 1.0)
            invc = scr.tile([D, 32], F, tag="invc")
            nc.vector.reciprocal(invc[:], denc[:])
            fcl = scr.tile([D, 32], F, tag="fcl")
            nc.vector.tensor_tensor(fcl[:], sqsc[:], invc[:], op=OP.mult)
            # d^2 = f*(f*sq - 2m) + q
            d2c = scr.tile([D, 32], F, tag="d2c")
            nc.vector.tensor_tensor(d2c[:], fcl[:], sqc[:], op=OP.mult)
            nc.vector.scalar_tensor_tensor(d2c[:], mc[:], -2.0, d2c[:],
                                           op0=OP.mult, op1=OP.add)
            nc.vector.tensor_tensor(d2c[:], d2c[:], fcl[:], op=OP.mult)
            nc.vector.tensor_tensor(d2c[:], d2c[:], q_pack[:], op=OP.add)
            d0p1 = row.tile([D, 1], F, tag="d0p1", name="d0p1")
            nc.scalar.activation(d0_pack[:], d2c[:], AF.Sqrt, accum_out=d0p1[:])

            # ================= routing helpers =================
            def row_t(tag):
                return row.tile([32, D], F, tag=tag, name=_nm(tag))

            def ar_start(tot, idx):
                """stage tot (1,1) -> DRAM -> AllReduce -> DMA result back"""
                stg = row.tile([1, 8], F, tag="arstg", name=_nm("arstg"))
                nc.vector.memset(stg[:], 0.0)
                nc.vector.tensor_copy(stg[0:1, 0:1], tot[:])
                ain = dram.tile([1, 8], F, tag=f"arin{idx}", name=_nm("dr_ain"))
                aout = dram.tile([1, 8], F, tag=f"arout{idx}",
                                 addr_space="Shared", name=_nm("dr_aout"))
                nc.sync.dma_start(ain[:], stg[:])
                if sim_mode:
                    nc.sync.dma_start(aout[:], ain[:])
                else:
                    nc.gpsimd.collective_compute(
                        "AllReduce", OP.add,
                        replica_groups=[list(range(N_CORES))],
                        ins=[ain.opt()], outs=[aout.opt()],
                    )
                gsum = row.tile([1, 1], F, tag="gsum", name=_nm("gsum"))
                nc.sync.dma_start(gsum[:], aout[0:1, 0:1])
                return gsum

            def ar_finish(gsum):
                """gsum -> t scalar -> (32,1) broadcast"""
                dent = row.tile([1, 1], F, tag="dent", name=_nm("dent"))
                nc.vector.tensor_scalar(dent[:], gsum[:], -0.5 / mean_div, EPS,
                                        op0=OP.mult, op1=OP.add)
                it = row.tile([1, 1], F, tag="it", name=_nm("it"))
                nc.vector.reciprocal(it[:], dent[:])
                tv = row.tile([1, 1], F, tag="tv", name=_nm("tv"))
                nc.vector.tensor_scalar_mul(tv[:], it[:], T_NUM)
                tb_ps = pss.tile([32, 1], F, tag="small", name=_nm("ps_tb"))
                nc.tensor.matmul(tb_ps[:], ones1x32[:], tv[:], start=True, stop=True)
                tb = row.tile([32, 1], F, tag="tb", name=_nm("tb"))
                nc.vector.tensor_copy(tb[:], tb_ps[:])
                return tb

            def softmax_c(d, tb):
                e = row_t("e")
                part = row.tile([32, 1], F, tag="epart", name=_nm("epart"))
                nc.scalar.activation(e[:], d[:], AF.Exp, scale=tb[:],
                                     accum_out=part[:])
                p4 = pss.tile([4, 1], F, tag="small", name=_nm("ps_p4"))
                nc.tensor.matmul(p4[:], m8[:], part[:], start=True, stop=True)
                s4 = row.tile([4, 1], F, tag="s4", name=_nm("s4"))
                nc.vector.tensor_copy(s4[:], p4[:])
                i4 = row.tile([4, 1], F, tag="i4", name=_nm("i4"))
                nc.vector.reciprocal(i4[:], s4[:])
                p32 = pss.tile([32, 1], F, tag="small", name=_nm("ps_p32"))
                nc.tensor.matmul(p32[:], m8t[:], i4[:], start=True, stop=True)
                inv32 = row.tile([32, 1], F, tag="inv32", name=_nm("inv32"))
                nc.vector.tensor_copy(inv32[:], p32[:])
                c = row_t("c")
                nc.vector.tensor_scalar(c[:], e[:], inv32[:], None, op0=OP.mult)
                return c

            def iter_sq(c):
                """sq = c(cq + 2rr) + bb; also m = cq + rr for d^2"""
                t1r = row_t("t1r")
                nc.vector.tensor_tensor(t1r[:], c[:], q_stack[:], op=OP.mult)
                m = row_t("m")
                nc.vector.tensor_tensor(m[:], t1r[:], rr_stack[:], op=OP.add)
                nc.vector.scalar_tensor_tensor(t1r[:], rr_stack[:], 2.0, t1r[:],
                                               op0=OP.mult, op1=OP.add)
                sq = row_t("sq")
                nc.vector.tensor_tensor(sq[:], c[:], t1r[:], op=OP.mult)
                nc.vector.tensor_tensor(sq[:], sq[:], bb32[:], op=OP.add)
                return sq, m

            def compute_f(sq):
                sqs = row_t("sqs")
                nc.scalar.activation(sqs[:], sq[:], AF.Sqrt)
                den = row_t("den")
                nc.vector.tensor_scalar_add(den[:], sq[:], 1.0)
                inv = row_t("invd")
                nc.vector.reciprocal(inv[:], den[:])
                f = row_t("f")
                nc.vector.tensor_tensor(f[:], sqs[:], inv[:], op=OP.mult)
                return f

            def compute_d_sum(f, m, sq):
                """d = sqrt(f(f sq - 2m) + q), plus fused row-sum of d"""
                d2 = row_t("d2")
                nc.vector.tensor_tensor(d2[:], f[:], sq[:], op=OP.mult)
                nc.vector.scalar_tensor_tensor(d2[:], m[:], -2.0, d2[:],
                                               op0=OP.mult, op1=OP.add)
                nc.vector.tensor_tensor(d2[:], d2[:], f[:], op=OP.mult)
                nc.vector.tensor_tensor(d2[:], d2[:], q_stack[:], op=OP.add)
                d = row_t("d")
                part = row.tile([32, 1], F, tag="dpart", name=_nm("dpart"))
                nc.scalar.activation(d[:], d2[:], AF.Sqrt, accum_out=part[:])
                tot = pss.tile([1, 1], F, tag="small", name=_nm("ps_tot"))
                nc.tensor.matmul(tot[:], part[:], ones32[:], start=True, stop=True)
                return d, tot

            # iteration 0: AllReduce the global d sum; fill the wait window
            # with layout transposes and the first gram tiles
            d0tot = pss.tile([1, 1], F, tag="small", name="ps_d0tot")
            nc.tensor.matmul(d0tot[:], d0p1[:], ones128[:], start=True, stop=True)
            gsum0 = ar_start(d0tot, 1)

            qs_ps = pss.tile([32, D], F, tag="small", name=_nm("ps_qs"))
            nc.tensor.transpose(qs_ps[:], q_pack[:], ident[:])
            q_stack = pp.tile([32, D], F)
            nc.vector.tensor_copy(q_stack[:], qs_ps[:])
            rs_ps = pss.tile([32, D], F, tag="small", name=_nm("ps_rs"))
            nc.tensor.transpose(rs_ps[:], rr_pack[:], ident[:])
            rr_stack = pp.tile([32, D], F)
            nc.vector.tensor_copy(rr_stack[:], rs_ps[:])
            d0r_ps = pss.tile([32, D], F, tag="small", name=_nm("ps_d0r"))
            nc.tensor.transpose(d0r_ps[:], d0_pack[:], ident[:])
            d = row_t("d0row")
            nc.vector.tensor_copy(d[:], d0r_ps[:])


            # ================= routing iterations =================
            tb = ar_finish(gsum0)
            # iteration 1
            c = softmax_c(d, tb)
            sq, m = iter_sq(c)
            f = compute_f(sq)
            d, tot = compute_d_sum(f, m, sq)
            gsum1 = ar_start(tot, 2)
            tb = ar_finish(gsum1)
            # iteration 2 (final): only need c, f
            c = softmax_c(d, tb)
            sq, _m = iter_sq(c)
            f = compute_f(sq)
            fc = row_t("fc")
            nc.vector.tensor_tensor(fc[:], f[:], c[:], op=OP.mult)

            # row -> column layout: (32,128) -> (128,32)
            fcT_ps = pss.tile([D, 32], F, tag="small", name=_nm("ps_fcT"))
            nc.tensor.transpose(fcT_ps[:], fc[:], ident[0:32, 0:32])
            fcT = pp.tile([D, 32], F)
            nc.vector.tensor_copy(fcT[:], fcT_ps[:])
            fT_ps = pss.tile([D, 32], F, tag="small", name=_nm("ps_fT"))
            nc.tensor.transpose(fT_ps[:], f[:], ident[0:32, 0:32])
            fT = pp.tile([D, 32], F)
            nc.vector.tensor_copy(fT[:], fT_ps[:])

            # evict coefficients for all batches at once
            acol = pp.tile([D, 32], F)
            nc.vector.tensor_tensor(acol[:], fcT[:], alpha_pack[:], op=OP.mult)
            ccol = pp.tile([D, 32], F)
            nc.vector.tensor_tensor(ccol[:], fT[:], bcol[:], op=OP.mult)

            # ===== phase 3: v = A*G + C from fp16 SBUF gram, stream out =====
            vt_cur = [None]
            for b in range(B_LOC):
                for ch in range(NCH):
                    g, h = ch // 2, ch % 2
                    if h == 0:
                        vt_cur[0] = vst.tile([D, 2 * N], F16, tag="vt",
                                             name=_nm("vt"))
                    vt = vt_cur[0]
                    dst = vt[:, N * h:N * (h + 1)]
                    k = 8 * b + ch
                    src = gsb[b * NCH + ch]
                    # DVE runs fp16 tiles at 2x; give it 3x the tiles of Act
                    if ch in (2, 6):
                        nc.scalar.activation(dst, src[:], AF.Identity,
                                             bias=ccol[:, k:k + 1],
                                             scale=acol[:, k:k + 1])
                    else:
                        nc.vector.tensor_scalar(dst, src[:], acol[:, k:k + 1],
                                                ccol[:, k:k + 1],
                                                op0=OP.mult, op1=OP.add)
                    if h == 1:
                        ring = (nc.sync, nc.gpsimd, nc.scalar)[(4 * b + g) % 3]
                        dst_ap = vout[b, 256 * g:256 * (g + 1), :].rearrange(
                            "(s p) n -> p s n", p=D)
                        src_ap = vt[:].rearrange("p (s n) -> p s n", n=N)
                        ring.dma_start(dst_ap, src_ap)

    nc.compile()
    return nc


def _get_nc():
    global _NC_CACHE
    if _NC_CACHE is None:
        _NC_CACHE = _build()
    return _NC_CACHE


def _reference_numpy(x, bias):
    """General fallback (non-row-constant bias): straight numpy port."""
    x = x.astype(np.float32)
    bias = bias.astype(np.float32)
    u_norm = np.linalg.norm(x, axis=1)[..., None]
    u_hat = np.einsum('bdn,bdm->bnm', x, x)
    u_hat_norm = np.linalg.norm(u_hat, axis=-1, keepdims=True)
    new_norm = np.minimum(u_hat_norm, u_norm)
    u_hat = u_hat / u_hat_norm * new_norm
    t_num = np.float32(T_NUM)
    b_ij = np.zeros(u_hat.shape, dtype=np.float32)
    v_j = None
    for it in range(3):
        m = b_ij.max(axis=1, keepdims=True)
        e = np.exp(b_ij - m)
        c_ij = e / e.sum(axis=1, keepdims=True)
        s_j = c_ij * u_hat + bias
        sqn = np.sum(s_j * s_j, axis=-1, keepdims=True)
        v_j = sqn * s_j / ((1.0 + sqn) * np.sqrt(sqn))
        if it < 2:
            dd = np.linalg.norm(v_j - u_hat, axis=-1, keepdims=True)
            d_o = dd.mean()
            t = t_num / (0.5 * d_o - d_o + EPS)
            b_ij = t * dd
    return v_j


def kernel(x, bias):
    global LAST_EXEC_NS
    x = np.ascontiguousarray(x, dtype=np.float32)
    bias = np.ascontiguousarray(bias, dtype=np.float32)
    B = x.shape[0]
    row_const = bool((bias == bias[:, :, :1]).all())
    if not row_const or B != 32 or x.shape[1:] != (D, N):
        return _reference_numpy(x, bias)
    brow = np.ascontiguousarray(bias[0, :, 0])  # (N,)
    xb16 = x.astype(ml_dtypes.bfloat16)
    # xT[b, p, 128c+j] = x[b, j, 128c+p]  (chunkwise transpose)
    xT16 = np.ascontiguousarray(
        x.reshape(B, D, NCH, D).transpose(0, 3, 2, 1)
    ).reshape(B, D, N).astype(ml_dtypes.bfloat16)
    sv = x.sum(axis=2).astype(ml_dtypes.bfloat16)  # (B, D) row sums
    # diag[b, p, c] col-layout: sum_d x[b,d,128c+p]^2 -> (B, 128, 8)
    dsq = (x * x).sum(axis=1).reshape(B, NCH, D).transpose(0, 2, 1)
    iden = np.eye(D, dtype=np.float32)
    m8 = np.zeros((32, 4), dtype=np.float32)
    m8t = np.zeros((4, 32), dtype=np.float32)
    for b in range(4):
        m8[8 * b:8 * b + 8, b] = 1.0
        m8t[b, 8 * b:8 * b + 8] = 1.0
    b32 = np.ascontiguousarray(np.tile(brow.reshape(8, 128), (4, 1)))
    bb32 = np.ascontiguousarray(np.float32(N) * b32 * b32)
    bcol = np.ascontiguousarray(np.tile(brow.reshape(8, 128).T, (1, 4)))
    bbcol = np.ascontiguousarray(np.float32(N) * bcol * bcol)
    nc = _get_nc()
    in_maps = [
        {"xb": np.ascontiguousarray(xb16[4 * c:4 * c + 4]),
         "xT": np.ascontiguousarray(xT16[4 * c:4 * c + 4]),
         "sv": np.ascontiguousarray(sv[4 * c:4 * c + 4].T),
         "diag": np.ascontiguousarray(
             dsq[4 * c:4 * c + 4].transpose(1, 0, 2).reshape(D, 32)),
         "iden": iden, "m8": m8, "m8t": m8t, "b32": b32, "bb32": bb32,
         "bcol": bcol, "bbcol": bbcol}
        for c in range(N_CORES)
    ]
    res = run_bass_kernel_spmd(nc, in_maps, core_ids=list(range(N_CORES)))
    LAST_EXEC_NS = res.exec_time_ns
    return np.concatenate(
        [res.results[c]["v"].astype(np.float32) for c in range(N_CORES)], axis=0)
